# revision 5
# baseline (speedup 1.0000x reference)
"""EventSampler Trainium2 kernel.

out[b,s,n] = exp_numbers[b,s,n,e*] where e* is the first e along the
trailing axis with unif[b,s,n,e] * rate[b,s] < total[b,s,n,e];
out = 5.0 (DTIME_MAX) when no e is accepted.

Strategy (pure data parallel over 8 NeuronCores, batch dim):
  per core shard: [2, 128, 32, 256] per big tensor.
  Layout: partitions = s (128), free = ns*NE within one batch row.
  Per [128, 256] row-tile, three fused DVE instructions:
    1. m   = (unif * rate) >= total              (scalar_tensor_tensor, chunk-wide)
    2. enc = (m + e/65536) * 65536 = m*65536+e;  idx = min(enc)
                                                 (tensor_tensor_reduce, accum=min)
    3. res = sum((enc == idx) * exp)             (scalar_tensor_tensor, accum=sum)
  idx >= 65536  <=>  nothing accepted -> overwrite res with 5.0.
"""

import numpy as np

import concourse.bacc as bacc
import concourse.mybir as mybir
import concourse.tile as tile
from concourse.bass_utils import run_bass_kernel_spmd

B, S, NS, NE = 16, 128, 32, 256
NCORES = 8
BL = B // NCORES  # batches per core
CH = 8            # ns rows per DMA chunk
BIGV = 65536.0
DTIME_MAX = 5.0
F32 = mybir.dt.float32

_CACHE = {}


def _build():
    nc = bacc.Bacc("TRN2", target_bir_lowering=False, debug=False, num_devices=NCORES)

    unif = nc.dram_tensor("unif", [BL, S, NS, NE], F32, kind="ExternalInput").ap()
    rate = nc.dram_tensor("rate", [BL, S], F32, kind="ExternalInput").ap()
    total = nc.dram_tensor("total", [BL, S, NS, NE], F32, kind="ExternalInput").ap()
    expn = nc.dram_tensor("expn", [BL, S, NS, NE], F32, kind="ExternalInput").ap()
    out = nc.dram_tensor("out", [BL, S, NS], F32, kind="ExternalOutput").ap()

    with tile.TileContext(nc) as tc:
        with (
            tc.tile_pool(name="const", bufs=1) as const_pool,
            tc.tile_pool(name="inp", bufs=2) as in_pool,
        ):
            # Constants: repeated iota table (0..255 per ns row), rate columns,
            # 5.0 fill.
            iotar = const_pool.tile([S, CH * NE], F32)
            nc.gpsimd.iota(
                iotar[:],
                pattern=[[0, CH], [1, NE]],
                base=0,
                channel_multiplier=0,
                allow_small_or_imprecise_dtypes=True,
            )

            rt = const_pool.tile([S, BL], F32)
            for b in range(BL):
                nc.sync.dma_start(rt[:, b : b + 1], rate[b, :].unsqueeze(1))

            fives = const_pool.tile([S, BL * NS], F32)
            nc.vector.memset(fives[:], DTIME_MAX)

            acc = const_pool.tile([S, BL * NS], F32)   # first-accept idx per row
            resc = const_pool.tile([S, BL * NS], F32)  # gathered exp per row

            for b in range(BL):
                for c in range(NS // CH):
                    ns0 = c * CH
                    ut = in_pool.tile([S, CH * NE], F32, tag="ut")
                    tt = in_pool.tile([S, CH * NE], F32, tag="tt")
                    et = in_pool.tile([S, CH * NE], F32, tag="et")
                    src = lambda t: t[b, :, ns0 : ns0 + CH, :].rearrange(
                        "s c e -> s (c e)"
                    )
                    nc.sync.dma_start(ut[:], src(unif))
                    nc.sync.dma_start(tt[:], src(total))
                    nc.sync.dma_start(et[:], src(expn))

                    # pass 1 (chunk-wide): m = (unif*rate) >= total, in place.
                    nc.vector.scalar_tensor_tensor(
                        out=ut[:],
                        in0=ut[:],
                        scalar=rt[:, b : b + 1],
                        in1=tt[:],
                        op0=mybir.AluOpType.mult,
                        op1=mybir.AluOpType.is_ge,
                    )
                    # pass 2 (chunk-wide): enc = m*65536 + e (into tt).
                    nc.vector.scalar_tensor_tensor(
                        out=tt[:],
                        in0=ut[:],
                        scalar=BIGV,
                        in1=iotar[:],
                        op0=mybir.AluOpType.mult,
                        op1=mybir.AluOpType.add,
                    )
                    # pass 3 (chunk-wide, segmented): idx = min(enc) per ns row.
                    col0 = b * NS + ns0
                    nc.vector.tensor_reduce(
                        out=acc[:, col0 : col0 + CH],
                        in_=tt[:].rearrange("s (c e) -> s c e", e=NE),
                        axis=mybir.AxisListType.X,
                        op=mybir.AluOpType.min,
                    )
                    for j in range(CH):
                        sl = slice(j * NE, (j + 1) * NE)
                        col = col0 + j
                        # pass 4: res = sum((enc == idx) * exp).
                        nc.vector.scalar_tensor_tensor(
                            out=ut[:, sl],
                            in0=tt[:, sl],
                            scalar=acc[:, col : col + 1],
                            in1=et[:, sl],
                            op0=mybir.AluOpType.is_equal,
                            op1=mybir.AluOpType.mult,
                            accum_out=resc[:, col : col + 1],
                        )

            # Fallback: rows with idx >= 65536 had no accepted sample.
            ge = const_pool.tile([S, BL * NS], mybir.dt.int32)
            nc.vector.tensor_scalar(
                out=ge[:],
                in0=acc[:],
                scalar1=BIGV,
                scalar2=None,
                op0=mybir.AluOpType.is_ge,
            )
            nc.vector.copy_predicated(resc[:], ge[:], fives[:])

            for b in range(BL):
                nc.sync.dma_start(out[b, :, :], resc[:, b * NS : (b + 1) * NS])

    nc.compile()
    return nc


def _get_nc():
    if "nc" not in _CACHE:
        _CACHE["nc"] = _build()
    return _CACHE["nc"]


def _make_in_maps(unif_numbers, sample_rate, total_intensities, exp_numbers):
    unif_numbers = np.ascontiguousarray(np.asarray(unif_numbers, dtype=np.float32))
    sample_rate = np.ascontiguousarray(np.asarray(sample_rate, dtype=np.float32))
    total_intensities = np.ascontiguousarray(
        np.asarray(total_intensities, dtype=np.float32)
    )
    exp_numbers = np.ascontiguousarray(np.asarray(exp_numbers, dtype=np.float32))
    in_maps = []
    for c in range(NCORES):
        sl = slice(c * BL, (c + 1) * BL)
        in_maps.append(
            {
                "unif": unif_numbers[sl],
                "rate": sample_rate[sl],
                "total": total_intensities[sl],
                "expn": exp_numbers[sl],
            }
        )
    return in_maps


def _run(in_maps, **kwargs):
    res = run_bass_kernel_spmd(_get_nc(), in_maps, list(range(NCORES)), **kwargs)
    out = np.concatenate([res.results[c]["out"] for c in range(NCORES)], axis=0)
    return out, res


def kernel(unif_numbers, sample_rate, total_intensities, exp_numbers):
    in_maps = _make_in_maps(
        unif_numbers, sample_rate, total_intensities, exp_numbers
    )
    out, _ = _run(in_maps)
    return out


# revision 6
# speedup vs baseline: 1.7654x; 1.7654x over previous
"""EventSampler Trainium2 kernel.

out[b,s,n] = exp_numbers[b,s,n,e*] where e* is the first e along the
trailing axis with unif[b,s,n,e] * rate[b,s] < total[b,s,n,e];
out = 5.0 (DTIME_MAX) when no e is accepted.

Strategy (pure data parallel over 8 NeuronCores on the batch dim):

Fast path ("prefix kernel"): for these acceptance probabilities the
first accepted index is tiny (P(first index >= 64) ~ 2^-64 per row), so
each core only receives the first K=64 columns of the three big tensors
(sliced host-side so the device DMAs are contiguous). Per [S, NS*K]
batch row, six chunk-wide DVE instructions:
    1. m   = (unif * rate) >= total          (scalar_tensor_tensor)
    2. enc = m*65536 + e                     (scalar_tensor_tensor)
    3. idx = min(enc) per ns-row             (segmented tensor_reduce)
    4. eq  = (enc == idx)                    (tensor_tensor, idx bcast)
    5. g   = eq * exp                        (tensor_tensor)
    6. res = sum(g) per ns-row               (segmented tensor_reduce)
idx >= 65536 marks a row with no accept within the prefix; the kernel
returns idx alongside the result and the host falls back to the full
256-column kernel (exact for arbitrary inputs) if any row is marked.
Both paths are bit-exact vs the reference formula.
"""

import numpy as np

import concourse.bacc as bacc
import concourse.mybir as mybir
import concourse.tile as tile
from concourse.bass_utils import run_bass_kernel_spmd

B, S, NS, NE = 16, 128, 32, 256
NCORES = 8
BL = B // NCORES  # batches per core
K = 64            # prefix columns scanned by the fast path
CH = 8            # ns rows per DMA chunk (full kernel)
BIGV = 65536.0
DTIME_MAX = 5.0
F32 = mybir.dt.float32

_CACHE = {}


def _build_fast():
    nc = bacc.Bacc("TRN2", target_bir_lowering=False, debug=False, num_devices=NCORES)

    unif = nc.dram_tensor("unif", [BL, S, NS, K], F32, kind="ExternalInput").ap()
    rate = nc.dram_tensor("rate", [BL, S], F32, kind="ExternalInput").ap()
    total = nc.dram_tensor("total", [BL, S, NS, K], F32, kind="ExternalInput").ap()
    expn = nc.dram_tensor("expn", [BL, S, NS, K], F32, kind="ExternalInput").ap()
    out = nc.dram_tensor("out", [BL, S, NS], F32, kind="ExternalOutput").ap()
    idxo = nc.dram_tensor("idxo", [BL, S, NS], F32, kind="ExternalOutput").ap()

    with tile.TileContext(nc) as tc:
        with (
            tc.tile_pool(name="const", bufs=1) as const_pool,
            tc.tile_pool(name="inp", bufs=2) as in_pool,
        ):
            iotar = const_pool.tile([S, NS * K], F32)
            nc.gpsimd.iota(
                iotar[:],
                pattern=[[0, NS], [1, K]],
                base=0,
                channel_multiplier=0,
                allow_small_or_imprecise_dtypes=True,
            )
            rt = const_pool.tile([S, BL], F32)
            for b in range(BL):
                nc.sync.dma_start(rt[:, b : b + 1], rate[b, :].unsqueeze(1))
            fives = const_pool.tile([S, BL * NS], F32)
            nc.vector.memset(fives[:], DTIME_MAX)
            acc = const_pool.tile([S, BL * NS], F32)
            resc = const_pool.tile([S, BL * NS], F32)

            for b in range(BL):
                ut = in_pool.tile([S, NS * K], F32, tag="ut")
                tt = in_pool.tile([S, NS * K], F32, tag="tt")
                et = in_pool.tile([S, NS * K], F32, tag="et")
                # Spread the three loads over the three DGE paths so they
                # stream concurrently.
                nc.sync.dma_start(ut[:], unif[b].rearrange("s c e -> s (c e)"))
                nc.scalar.dma_start(tt[:], total[b].rearrange("s c e -> s (c e)"))
                nc.gpsimd.dma_start(et[:], expn[b].rearrange("s c e -> s (c e)"))

                cols = slice(b * NS, (b + 1) * NS)
                # 1: m = (unif*rate) >= total, in place over unif.
                nc.vector.scalar_tensor_tensor(
                    out=ut[:],
                    in0=ut[:],
                    scalar=rt[:, b : b + 1],
                    in1=tt[:],
                    op0=mybir.AluOpType.mult,
                    op1=mybir.AluOpType.is_ge,
                )
                # 2: enc = m*65536 + e, over total.
                nc.vector.scalar_tensor_tensor(
                    out=tt[:],
                    in0=ut[:],
                    scalar=BIGV,
                    in1=iotar[:],
                    op0=mybir.AluOpType.mult,
                    op1=mybir.AluOpType.add,
                )
                # 3: idx = min(enc) per ns-row.
                nc.vector.tensor_reduce(
                    out=acc[:, cols],
                    in_=tt[:].rearrange("s (c e) -> s c e", e=K),
                    axis=mybir.AxisListType.X,
                    op=mybir.AluOpType.min,
                )
                # 4: eq = (enc == idx), idx broadcast along e.
                nc.vector.tensor_tensor(
                    out=ut[:].rearrange("s (c e) -> s c e", e=K),
                    in0=tt[:].rearrange("s (c e) -> s c e", e=K),
                    in1=acc[:, cols].unsqueeze(2).broadcast_to([S, NS, K]),
                    op=mybir.AluOpType.is_equal,
                )
                # 5: g = eq * exp, in place over exp.
                nc.vector.tensor_tensor(
                    out=et[:], in0=ut[:], in1=et[:], op=mybir.AluOpType.mult
                )
                # 6: res = sum(g) per ns-row.
                nc.vector.tensor_reduce(
                    out=resc[:, cols],
                    in_=et[:].rearrange("s (c e) -> s c e", e=K),
                    axis=mybir.AxisListType.X,
                    op=mybir.AluOpType.add,
                )

            ge = const_pool.tile([S, BL * NS], mybir.dt.int32)
            nc.vector.tensor_scalar(
                out=ge[:],
                in0=acc[:],
                scalar1=BIGV,
                scalar2=None,
                op0=mybir.AluOpType.is_ge,
            )
            nc.vector.copy_predicated(resc[:], ge[:], fives[:])

            for b in range(BL):
                cols = slice(b * NS, (b + 1) * NS)
                nc.sync.dma_start(out[b, :, :], resc[:, cols])
                nc.sync.dma_start(idxo[b, :, :], acc[:, cols])

    nc.compile()
    return nc


def _build_full():
    """Exact full-width kernel (fallback; also correct standalone)."""
    nc = bacc.Bacc("TRN2", target_bir_lowering=False, debug=False, num_devices=NCORES)

    unif = nc.dram_tensor("unif", [BL, S, NS, NE], F32, kind="ExternalInput").ap()
    rate = nc.dram_tensor("rate", [BL, S], F32, kind="ExternalInput").ap()
    total = nc.dram_tensor("total", [BL, S, NS, NE], F32, kind="ExternalInput").ap()
    expn = nc.dram_tensor("expn", [BL, S, NS, NE], F32, kind="ExternalInput").ap()
    out = nc.dram_tensor("out", [BL, S, NS], F32, kind="ExternalOutput").ap()

    with tile.TileContext(nc) as tc:
        with (
            tc.tile_pool(name="const", bufs=1) as const_pool,
            tc.tile_pool(name="inp", bufs=2) as in_pool,
        ):
            iotar = const_pool.tile([S, CH * NE], F32)
            nc.gpsimd.iota(
                iotar[:],
                pattern=[[0, CH], [1, NE]],
                base=0,
                channel_multiplier=0,
                allow_small_or_imprecise_dtypes=True,
            )
            rt = const_pool.tile([S, BL], F32)
            for b in range(BL):
                nc.sync.dma_start(rt[:, b : b + 1], rate[b, :].unsqueeze(1))
            fives = const_pool.tile([S, BL * NS], F32)
            nc.vector.memset(fives[:], DTIME_MAX)
            acc = const_pool.tile([S, BL * NS], F32)
            resc = const_pool.tile([S, BL * NS], F32)

            for b in range(BL):
                for c in range(NS // CH):
                    ns0 = c * CH
                    ut = in_pool.tile([S, CH * NE], F32, tag="ut")
                    tt = in_pool.tile([S, CH * NE], F32, tag="tt")
                    et = in_pool.tile([S, CH * NE], F32, tag="et")
                    src = lambda t: t[b, :, ns0 : ns0 + CH, :].rearrange(
                        "s c e -> s (c e)"
                    )
                    nc.sync.dma_start(ut[:], src(unif))
                    nc.scalar.dma_start(tt[:], src(total))
                    nc.gpsimd.dma_start(et[:], src(expn))

                    nc.vector.scalar_tensor_tensor(
                        out=ut[:],
                        in0=ut[:],
                        scalar=rt[:, b : b + 1],
                        in1=tt[:],
                        op0=mybir.AluOpType.mult,
                        op1=mybir.AluOpType.is_ge,
                    )
                    nc.vector.scalar_tensor_tensor(
                        out=tt[:],
                        in0=ut[:],
                        scalar=BIGV,
                        in1=iotar[:],
                        op0=mybir.AluOpType.mult,
                        op1=mybir.AluOpType.add,
                    )
                    col0 = b * NS + ns0
                    nc.vector.tensor_reduce(
                        out=acc[:, col0 : col0 + CH],
                        in_=tt[:].rearrange("s (c e) -> s c e", e=NE),
                        axis=mybir.AxisListType.X,
                        op=mybir.AluOpType.min,
                    )
                    for j in range(CH):
                        sl = slice(j * NE, (j + 1) * NE)
                        col = col0 + j
                        nc.vector.scalar_tensor_tensor(
                            out=ut[:, sl],
                            in0=tt[:, sl],
                            scalar=acc[:, col : col + 1],
                            in1=et[:, sl],
                            op0=mybir.AluOpType.is_equal,
                            op1=mybir.AluOpType.mult,
                            accum_out=resc[:, col : col + 1],
                        )

            ge = const_pool.tile([S, BL * NS], mybir.dt.int32)
            nc.vector.tensor_scalar(
                out=ge[:],
                in0=acc[:],
                scalar1=BIGV,
                scalar2=None,
                op0=mybir.AluOpType.is_ge,
            )
            nc.vector.copy_predicated(resc[:], ge[:], fives[:])

            for b in range(BL):
                nc.sync.dma_start(out[b, :, :], resc[:, b * NS : (b + 1) * NS])

    nc.compile()
    return nc


def _get(name, builder):
    if name not in _CACHE:
        _CACHE[name] = builder()
    return _CACHE[name]


def _shard(arr):
    return [arr[c * BL : (c + 1) * BL] for c in range(NCORES)]


def _run_fast(unif_p, rate, total_p, expn_p, **kwargs):
    nc = _get("fast", _build_fast)
    in_maps = [
        {"unif": u, "rate": r, "total": t, "expn": e}
        for u, r, t, e in zip(
            _shard(unif_p), _shard(rate), _shard(total_p), _shard(expn_p)
        )
    ]
    res = run_bass_kernel_spmd(nc, in_maps, list(range(NCORES)), **kwargs)
    out = np.concatenate([res.results[c]["out"] for c in range(NCORES)], axis=0)
    idxo = np.concatenate([res.results[c]["idxo"] for c in range(NCORES)], axis=0)
    return out, idxo, res


def _run_full(unif, rate, total, expn, **kwargs):
    nc = _get("full", _build_full)
    in_maps = [
        {"unif": u, "rate": r, "total": t, "expn": e}
        for u, r, t, e in zip(_shard(unif), _shard(rate), _shard(total), _shard(expn))
    ]
    res = run_bass_kernel_spmd(nc, in_maps, list(range(NCORES)), **kwargs)
    out = np.concatenate([res.results[c]["out"] for c in range(NCORES)], axis=0)
    return out, res


def kernel(unif_numbers, sample_rate, total_intensities, exp_numbers):
    unif_numbers = np.asarray(unif_numbers, dtype=np.float32)
    sample_rate = np.ascontiguousarray(np.asarray(sample_rate, dtype=np.float32))
    total_intensities = np.asarray(total_intensities, dtype=np.float32)
    exp_numbers = np.asarray(exp_numbers, dtype=np.float32)

    unif_p = np.ascontiguousarray(unif_numbers[..., :K])
    total_p = np.ascontiguousarray(total_intensities[..., :K])
    expn_p = np.ascontiguousarray(exp_numbers[..., :K])

    out, idxo, _ = _run_fast(unif_p, sample_rate, total_p, expn_p)
    if (idxo >= BIGV).any():
        # Some row had no accepted sample within the first K columns —
        # rerun exactly over the full width.
        unif_numbers = np.ascontiguousarray(unif_numbers)
        total_intensities = np.ascontiguousarray(total_intensities)
        exp_numbers = np.ascontiguousarray(exp_numbers)
        out, _ = _run_full(
            unif_numbers, sample_rate, total_intensities, exp_numbers
        )
    return out


# revision 7
# speedup vs baseline: 2.9766x; 1.6861x over previous
"""EventSampler Trainium2 kernel.

out[b,s,n] = exp_numbers[b,s,n,e*] where e* is the first e along the
trailing axis with unif[b,s,n,e] * rate[b,s] < total[b,s,n,e];
out = 5.0 (DTIME_MAX) when no e is accepted.

Strategy (pure data parallel over 8 NeuronCores on the batch dim):

Fast path ("prefix kernel"): for these acceptance probabilities the
first accepted index is tiny (P(first index >= 64) ~ 2^-64 per row), so
each core only receives the first K=64 columns of the three big tensors
(sliced host-side so the device DMAs are contiguous). Per [S, NS*K]
batch row, six chunk-wide DVE instructions:
    1. m   = (unif * rate) >= total          (scalar_tensor_tensor)
    2. enc = m*65536 + e                     (scalar_tensor_tensor)
    3. idx = min(enc) per ns-row             (segmented tensor_reduce)
    4. eq  = (enc == idx)                    (tensor_tensor, idx bcast)
    5. g   = eq * exp                        (tensor_tensor)
    6. res = sum(g) per ns-row               (segmented tensor_reduce)
idx >= 65536 marks a row with no accept within the prefix; the kernel
returns idx alongside the result and the host falls back to the full
256-column kernel (exact for arbitrary inputs) if any row is marked.
Both paths are bit-exact vs the reference formula.
"""

import numpy as np

import concourse.bacc as bacc
import concourse.mybir as mybir
import concourse.tile as tile
from concourse.bass_utils import run_bass_kernel_spmd

B, S, NS, NE = 16, 128, 32, 256
NCORES = 8
BL = B // NCORES  # batches per core
K = 32            # prefix columns scanned by the fast path
CH = 8            # ns rows per DMA chunk (full kernel)
BIGV = 65536.0
DTIME_MAX = 5.0
F32 = mybir.dt.float32

_CACHE = {}


def _build_fast():
    nc = bacc.Bacc("TRN2", target_bir_lowering=False, debug=False, num_devices=NCORES)

    unif = nc.dram_tensor("unif", [BL, S, NS, K], F32, kind="ExternalInput").ap()
    rate = nc.dram_tensor("rate", [BL, S], F32, kind="ExternalInput").ap()
    total = nc.dram_tensor("total", [BL, S, NS, K], F32, kind="ExternalInput").ap()
    expn = nc.dram_tensor("expn", [BL, S, NS, K], F32, kind="ExternalInput").ap()
    out = nc.dram_tensor("out", [BL, S, NS], F32, kind="ExternalOutput").ap()
    idxo = nc.dram_tensor("idxo", [BL, S, NS], F32, kind="ExternalOutput").ap()

    with tile.TileContext(nc) as tc:
        with (
            tc.tile_pool(name="const", bufs=1) as const_pool,
            tc.tile_pool(name="inp", bufs=2) as in_pool,
        ):
            iotar = const_pool.tile([S, NS * K], F32)
            nc.gpsimd.iota(
                iotar[:],
                pattern=[[0, NS], [1, K]],
                base=0,
                channel_multiplier=0,
                allow_small_or_imprecise_dtypes=True,
            )
            rt = const_pool.tile([S, BL], F32)
            for b in range(BL):
                nc.sync.dma_start(rt[:, b : b + 1], rate[b, :].unsqueeze(1))
            fives = const_pool.tile([S, BL * NS], F32)
            nc.vector.memset(fives[:], DTIME_MAX)
            acc = const_pool.tile([S, BL * NS], F32)
            resc = const_pool.tile([S, BL * NS], F32)

            for b in range(BL):
                ut = in_pool.tile([S, NS * K], F32, tag="ut")
                tt = in_pool.tile([S, NS * K], F32, tag="tt")
                et = in_pool.tile([S, NS * K], F32, tag="et")
                # Spread the three loads over the three DGE paths so they
                # stream concurrently.
                nc.sync.dma_start(ut[:], unif[b].rearrange("s c e -> s (c e)"))
                nc.scalar.dma_start(tt[:], total[b].rearrange("s c e -> s (c e)"))
                nc.scalar.dma_start(et[:], expn[b].rearrange("s c e -> s (c e)"))

                cols = slice(b * NS, (b + 1) * NS)
                # 1: m = (unif*rate) >= total, in place over unif.
                nc.vector.scalar_tensor_tensor(
                    out=ut[:],
                    in0=ut[:],
                    scalar=rt[:, b : b + 1],
                    in1=tt[:],
                    op0=mybir.AluOpType.mult,
                    op1=mybir.AluOpType.is_ge,
                )
                # 2: enc = m*65536 + e, over total.
                nc.vector.scalar_tensor_tensor(
                    out=tt[:],
                    in0=ut[:],
                    scalar=BIGV,
                    in1=iotar[:],
                    op0=mybir.AluOpType.mult,
                    op1=mybir.AluOpType.add,
                )
                # 3: idx = min(enc) per ns-row.
                nc.vector.tensor_reduce(
                    out=acc[:, cols],
                    in_=tt[:].rearrange("s (c e) -> s c e", e=K),
                    axis=mybir.AxisListType.X,
                    op=mybir.AluOpType.min,
                )
                # 4: eq = (enc == idx), idx broadcast along e.
                nc.vector.tensor_tensor(
                    out=ut[:].rearrange("s (c e) -> s c e", e=K),
                    in0=tt[:].rearrange("s (c e) -> s c e", e=K),
                    in1=acc[:, cols].unsqueeze(2).broadcast_to([S, NS, K]),
                    op=mybir.AluOpType.is_equal,
                )
                # 5: g = eq * exp, in place over exp.
                nc.vector.tensor_tensor(
                    out=et[:], in0=ut[:], in1=et[:], op=mybir.AluOpType.mult
                )
                # 6: res = sum(g) per ns-row.
                nc.vector.tensor_reduce(
                    out=resc[:, cols],
                    in_=et[:].rearrange("s (c e) -> s c e", e=K),
                    axis=mybir.AxisListType.X,
                    op=mybir.AluOpType.add,
                )

            ge = const_pool.tile([S, BL * NS], mybir.dt.int32)
            nc.vector.tensor_scalar(
                out=ge[:],
                in0=acc[:],
                scalar1=BIGV,
                scalar2=None,
                op0=mybir.AluOpType.is_ge,
            )
            nc.vector.copy_predicated(resc[:], ge[:], fives[:])

            for b in range(BL):
                cols = slice(b * NS, (b + 1) * NS)
                nc.sync.dma_start(out[b, :, :], resc[:, cols])
                nc.sync.dma_start(idxo[b, :, :], acc[:, cols])

    nc.compile()
    return nc


def _build_full():
    """Exact full-width kernel (fallback; also correct standalone)."""
    nc = bacc.Bacc("TRN2", target_bir_lowering=False, debug=False, num_devices=NCORES)

    unif = nc.dram_tensor("unif", [BL, S, NS, NE], F32, kind="ExternalInput").ap()
    rate = nc.dram_tensor("rate", [BL, S], F32, kind="ExternalInput").ap()
    total = nc.dram_tensor("total", [BL, S, NS, NE], F32, kind="ExternalInput").ap()
    expn = nc.dram_tensor("expn", [BL, S, NS, NE], F32, kind="ExternalInput").ap()
    out = nc.dram_tensor("out", [BL, S, NS], F32, kind="ExternalOutput").ap()

    with tile.TileContext(nc) as tc:
        with (
            tc.tile_pool(name="const", bufs=1) as const_pool,
            tc.tile_pool(name="inp", bufs=2) as in_pool,
        ):
            iotar = const_pool.tile([S, CH * NE], F32)
            nc.gpsimd.iota(
                iotar[:],
                pattern=[[0, CH], [1, NE]],
                base=0,
                channel_multiplier=0,
                allow_small_or_imprecise_dtypes=True,
            )
            rt = const_pool.tile([S, BL], F32)
            for b in range(BL):
                nc.sync.dma_start(rt[:, b : b + 1], rate[b, :].unsqueeze(1))
            fives = const_pool.tile([S, BL * NS], F32)
            nc.vector.memset(fives[:], DTIME_MAX)
            acc = const_pool.tile([S, BL * NS], F32)
            resc = const_pool.tile([S, BL * NS], F32)

            for b in range(BL):
                for c in range(NS // CH):
                    ns0 = c * CH
                    ut = in_pool.tile([S, CH * NE], F32, tag="ut")
                    tt = in_pool.tile([S, CH * NE], F32, tag="tt")
                    et = in_pool.tile([S, CH * NE], F32, tag="et")
                    src = lambda t: t[b, :, ns0 : ns0 + CH, :].rearrange(
                        "s c e -> s (c e)"
                    )
                    nc.sync.dma_start(ut[:], src(unif))
                    nc.scalar.dma_start(tt[:], src(total))
                    nc.gpsimd.dma_start(et[:], src(expn))

                    nc.vector.scalar_tensor_tensor(
                        out=ut[:],
                        in0=ut[:],
                        scalar=rt[:, b : b + 1],
                        in1=tt[:],
                        op0=mybir.AluOpType.mult,
                        op1=mybir.AluOpType.is_ge,
                    )
                    nc.vector.scalar_tensor_tensor(
                        out=tt[:],
                        in0=ut[:],
                        scalar=BIGV,
                        in1=iotar[:],
                        op0=mybir.AluOpType.mult,
                        op1=mybir.AluOpType.add,
                    )
                    col0 = b * NS + ns0
                    nc.vector.tensor_reduce(
                        out=acc[:, col0 : col0 + CH],
                        in_=tt[:].rearrange("s (c e) -> s c e", e=NE),
                        axis=mybir.AxisListType.X,
                        op=mybir.AluOpType.min,
                    )
                    for j in range(CH):
                        sl = slice(j * NE, (j + 1) * NE)
                        col = col0 + j
                        nc.vector.scalar_tensor_tensor(
                            out=ut[:, sl],
                            in0=tt[:, sl],
                            scalar=acc[:, col : col + 1],
                            in1=et[:, sl],
                            op0=mybir.AluOpType.is_equal,
                            op1=mybir.AluOpType.mult,
                            accum_out=resc[:, col : col + 1],
                        )

            ge = const_pool.tile([S, BL * NS], mybir.dt.int32)
            nc.vector.tensor_scalar(
                out=ge[:],
                in0=acc[:],
                scalar1=BIGV,
                scalar2=None,
                op0=mybir.AluOpType.is_ge,
            )
            nc.vector.copy_predicated(resc[:], ge[:], fives[:])

            for b in range(BL):
                nc.sync.dma_start(out[b, :, :], resc[:, b * NS : (b + 1) * NS])

    nc.compile()
    return nc


def _get(name, builder):
    if name not in _CACHE:
        _CACHE[name] = builder()
    return _CACHE[name]


def _shard(arr):
    return [arr[c * BL : (c + 1) * BL] for c in range(NCORES)]


def _run_fast(unif_p, rate, total_p, expn_p, **kwargs):
    nc = _get("fast", _build_fast)
    in_maps = [
        {"unif": u, "rate": r, "total": t, "expn": e}
        for u, r, t, e in zip(
            _shard(unif_p), _shard(rate), _shard(total_p), _shard(expn_p)
        )
    ]
    res = run_bass_kernel_spmd(nc, in_maps, list(range(NCORES)), **kwargs)
    out = np.concatenate([res.results[c]["out"] for c in range(NCORES)], axis=0)
    idxo = np.concatenate([res.results[c]["idxo"] for c in range(NCORES)], axis=0)
    return out, idxo, res


def _run_full(unif, rate, total, expn, **kwargs):
    nc = _get("full", _build_full)
    in_maps = [
        {"unif": u, "rate": r, "total": t, "expn": e}
        for u, r, t, e in zip(_shard(unif), _shard(rate), _shard(total), _shard(expn))
    ]
    res = run_bass_kernel_spmd(nc, in_maps, list(range(NCORES)), **kwargs)
    out = np.concatenate([res.results[c]["out"] for c in range(NCORES)], axis=0)
    return out, res


def kernel(unif_numbers, sample_rate, total_intensities, exp_numbers):
    unif_numbers = np.asarray(unif_numbers, dtype=np.float32)
    sample_rate = np.ascontiguousarray(np.asarray(sample_rate, dtype=np.float32))
    total_intensities = np.asarray(total_intensities, dtype=np.float32)
    exp_numbers = np.asarray(exp_numbers, dtype=np.float32)

    unif_p = np.ascontiguousarray(unif_numbers[..., :K])
    total_p = np.ascontiguousarray(total_intensities[..., :K])
    expn_p = np.ascontiguousarray(exp_numbers[..., :K])

    out, idxo, _ = _run_fast(unif_p, sample_rate, total_p, expn_p)
    if (idxo >= BIGV).any():
        # Some row had no accepted sample within the first K columns —
        # rerun exactly over the full width.
        unif_numbers = np.ascontiguousarray(unif_numbers)
        total_intensities = np.ascontiguousarray(total_intensities)
        exp_numbers = np.ascontiguousarray(exp_numbers)
        out, _ = _run_full(
            unif_numbers, sample_rate, total_intensities, exp_numbers
        )
    return out
